# revision 28
# baseline (speedup 1.0000x reference)
"""Causal self-attention Trainium2 kernel.

Full-input contract: kernel(x[4,2048,1024], w_qkv[1024,3072], w_proj[1024,1024])
-> [4,2048,1024] fp32.

Sharding (8 cores): batch (4) x head-group (2 groups of 8 heads).
Each core computes, for its (batch b, head-group g):
  - QKV^T projection for its 8 heads (tensor parallel on qkv columns)
  - causal attention for 8 heads, flash-style in S^T = K @ Q^T layout
  - partial out-projection (tensor parallel on proj rows)
Host sums the two partial Y contributions per batch (the "all-reduce").

Per-core layouts (T=2048, C=1024, D=64, 8 local heads):
  xT     [1024, 2048]  x[b].T             (bf16, host-transposed)
  wqkv   [1024, 1536]  [Wq|Wk|Wv] group-g column shards (bf16)
  wproj  [512, 1024]   group-g row shard (bf16)
  QT/KT  [512, 2048]   per-head-pair SBUF tiles [128, 2048]
  V      16 t-tiles [128, 520] = 8 x ([128,64] V_h | ones column)
  S^T    [k=128, q=512] psum tiles = K_tile @ Q_chunk  (2-head row-packed PE)
  E^T    exp(S^T/8) bf16, causal-masked via gpsimd affine_select
  O^T    [65, 512] psum = [V_h|1].T @ E^T  (row 64 = softmax denominators)
  Y      [2048, 1024] fp32 partial, DMA'd straight from PSUM
"""

import numpy as np
import ml_dtypes

import concourse.bass as bass
import concourse.bacc as bacc
import concourse.mybir as mybir
import concourse.tile as tile
from concourse.bass_utils import run_bass_kernel_spmd

B, T, C = 4, 2048, 1024
NH, D = 16, 64
HL = NH // 2          # heads per core
QC = 512              # q chunk (psum free dim)
KT = 128              # k tile (psum partitions)
NQC = T // QC         # 4 q chunks
NCT = C // KT         # 8 contraction tiles of 128
BF16 = mybir.dt.bfloat16
F32 = mybir.dt.float32

_CACHE = {}


def _build_nc():
    nc = bacc.Bacc("TRN2", target_bir_lowering=False, debug=False)
    xT = nc.dram_tensor("xT", [C, T], BF16, kind="ExternalInput")
    wqkv = nc.dram_tensor("wqkv", [C, 3 * HL * D], BF16, kind="ExternalInput")
    wproj = nc.dram_tensor("wproj", [HL * D, C], BF16, kind="ExternalInput")
    y = nc.dram_tensor("y", [T, C], F32, kind="ExternalOutput")

    with tile.TileContext(nc) as tc:
        with (
            tc.tile_pool(name="xt", bufs=1) as xt_pool,
            tc.tile_pool(name="wq", bufs=1) as wq_pool,
            tc.tile_pool(name="wp", bufs=1) as wp_pool,
            tc.tile_pool(name="qt", bufs=1) as qt_pool,
            tc.tile_pool(name="kt", bufs=1) as kt_pool,
            tc.tile_pool(name="vt", bufs=1) as vt_pool,
            tc.tile_pool(name="et", bufs=8) as et_pool,
            tc.tile_pool(name="mk", bufs=1) as mk_pool,
            tc.tile_pool(name="on", bufs=1) as on_pool,
            tc.tile_pool(name="ou", bufs=8) as ou_pool,
            tc.tile_pool(name="sm", bufs=4) as sm_pool,
            tc.tile_pool(name="bc", bufs=2) as bc_pool,
            tc.tile_pool(name="ys", bufs=3) as ys_pool,
            tc.tile_pool(name="sp", bufs=2, space="PSUM") as s_psum,
            tc.tile_pool(name="op", bufs=3, space="PSUM") as o_psum,
            tc.tile_pool(name="yp", bufs=1, space="PSUM") as y_psum,
        ):
            # ---- load inputs ----
            xt_sb = []
            for ci in range(NCT):
                t_ = xt_pool.tile([128, T], BF16, name=f"xt{ci}")
                nc.sync.dma_start(t_[:], xT[ci * 128:(ci + 1) * 128, :])
                xt_sb.append(t_)
            wq_sb = []
            for ci in range(NCT):
                t_ = wq_pool.tile([128, 3 * HL * D], BF16, name=f"wq{ci}")
                nc.sync.dma_start(t_[:], wqkv[ci * 128:(ci + 1) * 128, :])
                wq_sb.append(t_)
            wp_sb = []
            for ci in range(4):
                t_ = wp_pool.tile([128, C], BF16, name=f"wp{ci}")
                nc.sync.dma_start(t_[:], wproj[ci * 128:(ci + 1) * 128, :])
                wp_sb.append(t_)

            # ---- phase 1b: V tiles [128, 520] with ones columns,
            # emitted per-chunk below (chunk j needs tiles 4j..4j+3)
            v_sb = [None] * (T // KT)

            def emit_v(tt):
                vt = vt_pool.tile([128, HL * (D + 1)], BF16, name=f"vt{tt}")
                ps = s_psum.tile([128, QC], F32, name="ps", tag="sp")
                for ci in range(NCT):
                    nc.tensor.matmul(
                        ps[:],
                        lhsT=xt_sb[ci][:, tt * 128:(tt + 1) * 128],
                        rhs=wq_sb[ci][:, 2 * HL * D:3 * HL * D],
                        start=(ci == 0),
                        stop=(ci == NCT - 1),
                    )
                nc.vector.tensor_copy(
                    vt[:].rearrange("p (h e) -> p h e", e=D + 1)[:, :, 0:D],
                    ps[:].rearrange("p (h e) -> p h e", e=D),
                )
                nc.gpsimd.memset(
                    vt[:].rearrange("p (h e) -> p h e", e=D + 1)[:, :, D:D + 1],
                    1.0,
                )
                v_sb[tt] = vt

            # ---- phase 1a: Q^T, K^T  [512,2048] each as 4 pair-tiles ----
            # emitted per t-chunk (interleaved with attention below)
            qt_sb = [qt_pool.tile([128, T], BF16, name=f"qt{i}") for i in range(4)]
            kt_sb = [kt_pool.tile([128, T], BF16, name=f"kts{i}") for i in range(4)]

            def emit_qk(tch):
                for ct in range(8):  # c' tiles over [Q^T; K^T] rows (1024)
                    dst = qt_sb[ct] if ct < 4 else kt_sb[ct - 4]
                    ps = s_psum.tile([128, QC], F32, name="ps", tag="sp")
                    for ci in range(NCT):
                        nc.tensor.matmul(
                            ps[:],
                            lhsT=wq_sb[ci][:, ct * 128:(ct + 1) * 128],
                            rhs=xt_sb[ci][:, tch * QC:(tch + 1) * QC],
                            start=(ci == 0),
                            stop=(ci == NCT - 1),
                        )
                    nc.vector.tensor_copy(
                        dst[:, tch * QC:(tch + 1) * QC], ps[:]
                    )

            # ---- phase 2: attention + out-proj, per q-chunk ----
            # Heads run per pair (2 heads sharing a QT/KT tile). Per k-tile,
            # both heads' S^T land in one [128,1024] PSUM tile (row-packed
            # concurrent matmuls via tile_position), one 1024-wide exp, then
            # two AV accumulations. O^T leaves PSUM immediately (unnormal-
            # ized); normalization happens later from SBUF.
            on_sb = [on_pool.tile([128, T], BF16, name=f"on{i}") for i in range(4)]

            def emit_skt(j, pair, kt_i, nk, ops):
                sp = s_psum.tile([128, 2 * QC], F32, name="sp", tag="sp")
                for slot in range(2):
                    po = slot * 64
                    nc.tensor.matmul(
                        sp[:, slot * QC:(slot + 1) * QC],
                        lhsT=kt_sb[pair][po:po + 64, kt_i * KT:(kt_i + 1) * KT],
                        rhs=qt_sb[pair][po:po + 64, j * QC:(j + 1) * QC],
                        start=True,
                        stop=True,
                        tile_position=(po, 0),
                    )
                et = et_pool.tile([128, 2 * QC], BF16, name="et")
                nc.scalar.activation(
                    et[:], sp[:], mybir.ActivationFunctionType.Exp, scale=0.125
                )
                if kt_i >= 4 * j:  # diagonal-crossing tile
                    for half in range(2):
                        nc.gpsimd.affine_select(
                            out=et[:, half * QC:(half + 1) * QC],
                            in_=et[:, half * QC:(half + 1) * QC],
                            compare_op=mybir.AluOpType.is_ge,
                            fill=0.0,
                            base=j * QC - kt_i * KT,
                            pattern=[[1, QC]],
                            channel_multiplier=-1,
                        )
                for slot in range(2):
                    h = pair * 2 + slot
                    nc.tensor.matmul(
                        ops[slot][:],
                        lhsT=v_sb[kt_i][:, h * (D + 1):(h + 1) * (D + 1)],
                        rhs=et[:, slot * QC:(slot + 1) * QC],
                        start=(kt_i == 0),
                        stop=(kt_i == nk - 1),
                    )

            def emit_pair(j, pair, nk, sums4, ou_t):
                ops = [o_psum.tile([65, QC], F32, name=f"op{s}", tag="op")
                       for s in range(2)]
                for kt_i in range(nk):
                    emit_skt(j, pair, kt_i, nk, ops)
                for slot in range(2):
                    hh = (pair % 2) * 2 + slot
                    ou = ou_pool.tile([64, QC], BF16, name="ou")
                    nc.vector.tensor_copy(ou[:], ops[slot][0:64, :])
                    ou_t.append(ou)
                    nc.vector.tensor_copy(
                        sums4[32 * hh:32 * hh + 1, :], ops[slot][64:65, :]
                    )

            def emit_norm(j, half4, sums4, ou_t):
                recip4 = sm_pool.tile([97, QC], F32, name="recip4", tag="recip")
                nc.vector.reciprocal(recip4[:], sums4[:])
                for hh in range(4):
                    h = half4 * 4 + hh
                    rc1 = sm_pool.tile([1, QC], F32, name="rc1", tag="rc1")
                    nc.vector.tensor_copy(rc1[:], recip4[32 * hh:32 * hh + 1, :])
                    bc = bc_pool.tile([64, QC], F32, name="bc")
                    nc.gpsimd.partition_broadcast(bc[:], rc1[:])
                    nc.vector.tensor_mul(
                        on_sb[h // 2][(h % 2) * 64:(h % 2) * 64 + 64,
                                      j * QC:(j + 1) * QC],
                        ou_t[hh][:],
                        bc[:],
                    )

            def emit_proj(j):
                for qq in range(QC // 128):
                    qt0 = j * QC + qq * 128
                    for co in range(2):
                        yp = y_psum.tile([128, QC], F32, name="yp")
                        for ci2 in range(4):
                            nc.tensor.matmul(
                                yp[:],
                                lhsT=on_sb[ci2][:, qt0:qt0 + 128],
                                rhs=wp_sb[ci2][:, co * QC:(co + 1) * QC],
                                start=(ci2 == 0),
                                stop=(ci2 == 3),
                            )
                        yst = ys_pool.tile([128, QC], F32, name="yst")
                        nc.vector.tensor_copy(yst[:], yp[:])
                        nc.sync.dma_start(
                            y[qt0:qt0 + 128, co * QC:(co + 1) * QC], yst[:]
                        )

            for j in range(NQC):
                for tt in range(4 * j, 4 * j + 4):
                    emit_v(tt)
                emit_qk(j)
                nk = 4 * j + 4  # causal: k tiles 0..nk-1
                for half4 in range(2):  # two groups of 2 pairs each
                    sums4 = sm_pool.tile([97, QC], F32, name="sums4", tag="sums")
                    ou_t = []
                    for pp in range(2):
                        emit_pair(j, half4 * 2 + pp, nk, sums4, ou_t)
                    emit_norm(j, half4, sums4, ou_t)
                emit_proj(j)
    nc.finalize()
    return nc


def _shard_inputs(x, w_qkv, w_proj):
    bf = ml_dtypes.bfloat16
    in_maps = []
    for core in range(8):
        b, g = core // 2, core % 2
        cols = slice(g * HL * D, (g + 1) * HL * D)
        wq = np.ascontiguousarray(
            np.concatenate(
                [w_qkv[:, 0 * C:1 * C][:, cols],
                 w_qkv[:, 1 * C:2 * C][:, cols],
                 w_qkv[:, 2 * C:3 * C][:, cols]], axis=1
            ).astype(bf)
        )
        wp = np.ascontiguousarray(w_proj[g * HL * D:(g + 1) * HL * D, :].astype(bf))
        xt = np.ascontiguousarray(x[b].T.astype(bf))
        in_maps.append({"xT": xt, "wqkv": wq, "wproj": wp})
    return in_maps


def kernel(x, w_qkv, w_proj, trace=False, **trace_kwargs):
    if "nc" not in _CACHE:
        _CACHE["nc"] = _build_nc()
    nc = _CACHE["nc"]
    in_maps = _shard_inputs(
        np.asarray(x, np.float32), np.asarray(w_qkv, np.float32),
        np.asarray(w_proj, np.float32)
    )
    res = run_bass_kernel_spmd(
        nc, in_maps, core_ids=list(range(8)), trace=trace, **trace_kwargs
    )
    parts = [res.results[core]["y"] for core in range(8)]
    out = np.stack(
        [parts[2 * b].astype(np.float32) + parts[2 * b + 1].astype(np.float32)
         for b in range(B)]
    )
    if trace:
        _CACHE["last_result"] = res
    return out


# revision 29
# speedup vs baseline: 1.2206x; 1.2206x over previous
"""Causal self-attention Trainium2 kernel.

Full-input contract: kernel(x[4,2048,1024], w_qkv[1024,3072], w_proj[1024,1024])
-> [4,2048,1024] fp32.

Sharding (8 cores): batch (4) x head-group (2 groups of 8 heads).
Each core computes, for its (batch b, head-group g):
  - QKV^T projection for its 8 heads (tensor parallel on qkv columns)
  - causal attention for 8 heads, flash-style in S^T = K @ Q^T layout
  - partial out-projection (tensor parallel on proj rows)
Host sums the two partial Y contributions per batch (the "all-reduce").

Per-core layouts (T=2048, C=1024, D=64, 8 local heads):
  xT     [1024, 2048]  x[b].T             (bf16, host-transposed)
  wqkv   [1024, 1536]  [Wq|Wk|Wv] group-g column shards (bf16)
  wproj  [512, 1024]   group-g row shard (bf16)
  QT/KT  [512, 2048]   per-head-pair SBUF tiles [128, 2048]
  V      16 t-tiles [128, 520] = 8 x ([128,64] V_h | ones column)
  S^T    [k=128, q=512] psum tiles = K_tile @ Q_chunk  (2-head row-packed PE)
  E^T    exp(S^T/8) bf16, causal-masked via gpsimd affine_select
  O^T    [65, 512] psum = [V_h|1].T @ E^T  (row 64 = softmax denominators)
  Y      [2048, 1024] fp32 partial, DMA'd straight from PSUM
"""

import numpy as np
import ml_dtypes

import concourse.bass as bass
import concourse.bacc as bacc
import concourse.mybir as mybir
import concourse.tile as tile
from concourse.bass_utils import run_bass_kernel_spmd

B, T, C = 4, 2048, 1024
NH, D = 16, 64
HL = NH // 2          # heads per core
QC = 512              # q chunk (psum free dim)
KT = 128              # k tile (psum partitions)
NQC = T // QC         # 4 q chunks
NCT = C // KT         # 8 contraction tiles of 128
BF16 = mybir.dt.bfloat16
F32 = mybir.dt.float32

_CACHE = {}


def _build_nc():
    nc = bacc.Bacc("TRN2", target_bir_lowering=False, debug=False)
    xT = nc.dram_tensor("xT", [C, T], BF16, kind="ExternalInput")
    wqkv = nc.dram_tensor("wqkv", [C, 3 * HL * D], BF16, kind="ExternalInput")
    wproj = nc.dram_tensor("wproj", [HL * D, C], BF16, kind="ExternalInput")
    y = nc.dram_tensor("y", [T, C], F32, kind="ExternalOutput")

    with tile.TileContext(nc) as tc:
        with (
            tc.tile_pool(name="xt", bufs=1) as xt_pool,
            tc.tile_pool(name="wq", bufs=1) as wq_pool,
            tc.tile_pool(name="wp", bufs=1) as wp_pool,
            tc.tile_pool(name="qt", bufs=1) as qt_pool,
            tc.tile_pool(name="kt", bufs=1) as kt_pool,
            tc.tile_pool(name="vt", bufs=1) as vt_pool,
            tc.tile_pool(name="et", bufs=8) as et_pool,
            tc.tile_pool(name="mk", bufs=1) as mk_pool,
            tc.tile_pool(name="on", bufs=1) as on_pool,
            tc.tile_pool(name="ou", bufs=8) as ou_pool,
            tc.tile_pool(name="sm", bufs=4) as sm_pool,
            tc.tile_pool(name="bc", bufs=2) as bc_pool,
            tc.tile_pool(name="ys", bufs=3) as ys_pool,
            tc.tile_pool(name="sp", bufs=2, space="PSUM") as s_psum,
            tc.tile_pool(name="op", bufs=3, space="PSUM") as o_psum,
            tc.tile_pool(name="yp", bufs=1, space="PSUM") as y_psum,
        ):
            # ---- load inputs ----
            xt_sb = []
            for ci in range(NCT):
                t_ = xt_pool.tile([128, T], BF16, name=f"xt{ci}")
                nc.sync.dma_start(t_[:], xT[ci * 128:(ci + 1) * 128, :])
                xt_sb.append(t_)
            wq_sb = []
            for ci in range(NCT):
                t_ = wq_pool.tile([128, 3 * HL * D], BF16, name=f"wq{ci}")
                nc.sync.dma_start(t_[:], wqkv[ci * 128:(ci + 1) * 128, :])
                wq_sb.append(t_)
            wp_sb = []
            for ci in range(4):
                t_ = wp_pool.tile([128, C], BF16, name=f"wp{ci}")
                nc.sync.dma_start(t_[:], wproj[ci * 128:(ci + 1) * 128, :])
                wp_sb.append(t_)

            # ---- phase 1b: V tiles [128, 520] with ones columns,
            # emitted per-chunk below (chunk j needs tiles 4j..4j+3)
            v_sb = [None] * (T // KT)

            def emit_v(tt):
                vt = vt_pool.tile([128, HL * (D + 1)], BF16, name=f"vt{tt}")
                ps = s_psum.tile([128, QC], F32, name="ps", tag="sp")
                for ci in range(NCT):
                    nc.tensor.matmul(
                        ps[:],
                        lhsT=xt_sb[ci][:, tt * 128:(tt + 1) * 128],
                        rhs=wq_sb[ci][:, 2 * HL * D:3 * HL * D],
                        start=(ci == 0),
                        stop=(ci == NCT - 1),
                    )
                nc.vector.tensor_copy(
                    vt[:].rearrange("p (h e) -> p h e", e=D + 1)[:, :, 0:D],
                    ps[:].rearrange("p (h e) -> p h e", e=D),
                )
                nc.gpsimd.memset(
                    vt[:].rearrange("p (h e) -> p h e", e=D + 1)[:, :, D:D + 1],
                    1.0,
                )
                v_sb[tt] = vt

            # ---- phase 1a: Q^T, K^T  [512,2048] each as 4 pair-tiles ----
            # emitted per t-chunk (interleaved with attention below)
            qt_sb = [qt_pool.tile([128, T], BF16, name=f"qt{i}") for i in range(4)]
            kt_sb = [kt_pool.tile([128, T], BF16, name=f"kts{i}") for i in range(4)]

            def emit_qk(tch):
                for ct in range(8):  # c' tiles over [Q^T; K^T] rows (1024)
                    dst = qt_sb[ct] if ct < 4 else kt_sb[ct - 4]
                    ps = s_psum.tile([128, QC], F32, name="ps", tag="sp")
                    for ci in range(NCT):
                        nc.tensor.matmul(
                            ps[:],
                            lhsT=wq_sb[ci][:, ct * 128:(ct + 1) * 128],
                            rhs=xt_sb[ci][:, tch * QC:(tch + 1) * QC],
                            start=(ci == 0),
                            stop=(ci == NCT - 1),
                        )
                    nc.vector.tensor_copy(
                        dst[:, tch * QC:(tch + 1) * QC], ps[:]
                    )

            # ---- phase 2: attention + out-proj, per q-chunk ----
            # Heads run per pair (2 heads sharing a QT/KT tile). Per k-tile,
            # both heads' S^T land in one [128,1024] PSUM tile (row-packed
            # concurrent matmuls via tile_position), one 1024-wide exp, then
            # two AV accumulations. O^T leaves PSUM immediately (unnormal-
            # ized); normalization happens later from SBUF.
            # masks [128,1024] = [m|m]: keep (q - k >= 0) for diag tile m
            mask_sb = []
            for m in range(4):
                mk = mk_pool.tile([128, 2 * QC], BF16, name=f"mask{m}")
                nc.gpsimd.memset(mk[:], 1.0)
                for half in range(2):
                    nc.gpsimd.affine_select(
                        out=mk[:, half * QC:(half + 1) * QC],
                        in_=mk[:, half * QC:(half + 1) * QC],
                        compare_op=mybir.AluOpType.is_ge,
                        fill=0.0,
                        base=-m * KT,
                        pattern=[[1, QC]],
                        channel_multiplier=-1,
                    )
                mask_sb.append(mk)

            on_sb = [on_pool.tile([128, T], BF16, name=f"on{i}") for i in range(4)]

            def emit_skt(j, pair, kt_i, nk, ops):
                sp = s_psum.tile([128, 2 * QC], F32, name="sp", tag="sp")
                for slot in range(2):
                    po = slot * 64
                    nc.tensor.matmul(
                        sp[:, slot * QC:(slot + 1) * QC],
                        lhsT=kt_sb[pair][po:po + 64, kt_i * KT:(kt_i + 1) * KT],
                        rhs=qt_sb[pair][po:po + 64, j * QC:(j + 1) * QC],
                        start=True,
                        stop=True,
                        tile_position=(po, 0),
                    )
                et = et_pool.tile([128, 2 * QC], BF16, name="et")
                nc.scalar.activation(
                    et[:], sp[:], mybir.ActivationFunctionType.Exp, scale=0.125
                )
                if kt_i >= 4 * j:  # diagonal-crossing tile
                    nc.vector.tensor_mul(
                        et[:], et[:], mask_sb[kt_i - 4 * j][:]
                    )
                for slot in range(2):
                    h = pair * 2 + slot
                    nc.tensor.matmul(
                        ops[slot][:],
                        lhsT=v_sb[kt_i][:, h * (D + 1):(h + 1) * (D + 1)],
                        rhs=et[:, slot * QC:(slot + 1) * QC],
                        start=(kt_i == 0),
                        stop=(kt_i == nk - 1),
                    )

            def emit_pair(j, pair, nk, sums4, ou_t):
                ops = [o_psum.tile([65, QC], F32, name=f"op{s}", tag="op")
                       for s in range(2)]
                for kt_i in range(nk):
                    emit_skt(j, pair, kt_i, nk, ops)
                for slot in range(2):
                    hh = (pair % 2) * 2 + slot
                    ou = ou_pool.tile([64, QC], BF16, name="ou")
                    nc.vector.tensor_copy(ou[:], ops[slot][0:64, :])
                    ou_t.append(ou)
                    nc.vector.tensor_copy(
                        sums4[32 * hh:32 * hh + 1, :], ops[slot][64:65, :]
                    )

            def emit_norm(j, half4, sums4, ou_t):
                lns = sm_pool.tile([97, QC], F32, name="lns", tag="lns")
                nc.scalar.activation(
                    lns[:], sums4[:], mybir.ActivationFunctionType.Ln
                )
                recip4 = sm_pool.tile([97, QC], F32, name="recip4", tag="recip")
                nc.scalar.activation(
                    recip4[:], lns[:], mybir.ActivationFunctionType.Exp,
                    scale=-1.0,
                )
                for hh in range(4):
                    h = half4 * 4 + hh
                    rc1 = sm_pool.tile([1, QC], F32, name="rc1", tag="rc1")
                    nc.vector.tensor_copy(rc1[:], recip4[32 * hh:32 * hh + 1, :])
                    bc = bc_pool.tile([64, QC], F32, name="bc")
                    nc.gpsimd.partition_broadcast(bc[:], rc1[:])
                    nc.vector.tensor_mul(
                        on_sb[h // 2][(h % 2) * 64:(h % 2) * 64 + 64,
                                      j * QC:(j + 1) * QC],
                        ou_t[hh][:],
                        bc[:],
                    )

            def emit_proj(j):
                for qq in range(QC // 128):
                    qt0 = j * QC + qq * 128
                    for co in range(2):
                        yp = y_psum.tile([128, QC], F32, name="yp")
                        for ci2 in range(4):
                            nc.tensor.matmul(
                                yp[:],
                                lhsT=on_sb[ci2][:, qt0:qt0 + 128],
                                rhs=wp_sb[ci2][:, co * QC:(co + 1) * QC],
                                start=(ci2 == 0),
                                stop=(ci2 == 3),
                            )
                        yst = ys_pool.tile([128, QC], F32, name="yst")
                        nc.vector.tensor_copy(yst[:], yp[:])
                        nc.sync.dma_start(
                            y[qt0:qt0 + 128, co * QC:(co + 1) * QC], yst[:]
                        )

            for j in range(NQC):
                for tt in range(4 * j, 4 * j + 4):
                    emit_v(tt)
                emit_qk(j)
                nk = 4 * j + 4  # causal: k tiles 0..nk-1
                for half4 in range(2):  # two groups of 2 pairs each
                    sums4 = sm_pool.tile([97, QC], F32, name="sums4", tag="sums")
                    ou_t = []
                    for pp in range(2):
                        emit_pair(j, half4 * 2 + pp, nk, sums4, ou_t)
                    emit_norm(j, half4, sums4, ou_t)
                emit_proj(j)
    nc.finalize()
    return nc


def _shard_inputs(x, w_qkv, w_proj):
    bf = ml_dtypes.bfloat16
    in_maps = []
    for core in range(8):
        b, g = core // 2, core % 2
        cols = slice(g * HL * D, (g + 1) * HL * D)
        wq = np.ascontiguousarray(
            np.concatenate(
                [w_qkv[:, 0 * C:1 * C][:, cols],
                 w_qkv[:, 1 * C:2 * C][:, cols],
                 w_qkv[:, 2 * C:3 * C][:, cols]], axis=1
            ).astype(bf)
        )
        wp = np.ascontiguousarray(w_proj[g * HL * D:(g + 1) * HL * D, :].astype(bf))
        xt = np.ascontiguousarray(x[b].T.astype(bf))
        in_maps.append({"xT": xt, "wqkv": wq, "wproj": wp})
    return in_maps


def kernel(x, w_qkv, w_proj, trace=False, **trace_kwargs):
    if "nc" not in _CACHE:
        _CACHE["nc"] = _build_nc()
    nc = _CACHE["nc"]
    in_maps = _shard_inputs(
        np.asarray(x, np.float32), np.asarray(w_qkv, np.float32),
        np.asarray(w_proj, np.float32)
    )
    res = run_bass_kernel_spmd(
        nc, in_maps, core_ids=list(range(8)), trace=trace, **trace_kwargs
    )
    parts = [res.results[core]["y"] for core in range(8)]
    out = np.stack(
        [parts[2 * b].astype(np.float32) + parts[2 * b + 1].astype(np.float32)
         for b in range(B)]
    )
    if trace:
        _CACHE["last_result"] = res
    return out
